# revision 1
# baseline (speedup 1.0000x reference)
"""AttentionBlock (GroupNorm + 1x1 QKV + MHA + proj + residual) on 8 trn2 cores.

Sharding: core c -> (batch b = c//2, t-half = c%2). Each core computes all 4
heads for its 2048 query positions; k/v are computed over the full T=4096 from
the core's batch. No cross-core communication needed.

Layout trick: attention scores are computed TRANSPOSED, S_T[s, t] (s on
partitions), so the AV matmul needs no transposes: a[ch, t] = vT[s, ch].T @
P_T[s, t]. The softmax denominator comes from a ones-column appended to vT.
GroupNorm is folded into the QKV weights (per-channel scale/shift).
"""

import math

import numpy as np

import concourse.bass as bass
import concourse.tile as tile
from concourse import bacc, mybir
from concourse import bass_utils

F32 = mybir.dt.float32
F32R = mybir.dt.float32r
BF16 = mybir.dt.bfloat16
F16 = mybir.dt.float16

B, C, HH, WW = 4, 256, 64, 64
T = HH * WW            # 4096
NH = 4                 # heads per batch
CH = C // NH           # 64 channels per head
G = 32                 # groupnorm groups
CPG = C // G           # 8 channels per group
EPS = 1e-5
NCORES = 8
THALF = T // 2         # 2048  t-columns per core
SCALE = 1.0 / math.sqrt(math.sqrt(CH))

USE_F32R = True        # f32r (tf32-like) for the big matmuls
P_DT = BF16            # dtype of exp'd attention weights + vT (bf16 or f32)

_CACHE = {}
LAST_RESULTS = None




def _r(ap):
    return ap


def _build_program():
    nc = bacc.Bacc("TRN2", target_bir_lowering=False, debug=False)

    d_xb = nc.dram_tensor("xb", [C, T], F16, kind="ExternalInput").ap()
    d_xq = nc.dram_tensor("xq", [C, THALF], F16, kind="ExternalInput").ap()
    d_xqf = nc.dram_tensor("xqf", [C, THALF], F32, kind="ExternalInput").ap()
    d_wT = nc.dram_tensor("wT", [C, 3 * C], F32, kind="ExternalInput").ap()
    d_qkvb = nc.dram_tensor("qkvb", [3 * C, 1], F32, kind="ExternalInput").ap()
    d_vbrow = nc.dram_tensor("vbrow", [1, C], F32, kind="ExternalInput").ap()
    d_pjT = nc.dram_tensor("pjT", [C, C], F16, kind="ExternalInput").ap()
    d_pjb = nc.dram_tensor("pjb", [C, 1], F32, kind="ExternalInput").ap()
    d_gnw = nc.dram_tensor("gnw", [C, 1], F32, kind="ExternalInput").ap()
    d_gnb = nc.dram_tensor("gnb", [C, 1], F32, kind="ExternalInput").ap()
    d_gsel = nc.dram_tensor("gsel", [128, 2 * G], F32, kind="ExternalInput").ap()
    d_bsel = nc.dram_tensor("bsel", [G, C], F32, kind="ExternalInput").ap()
    d_out = nc.dram_tensor("out", [C, THALF], F32, kind="ExternalOutput").ap()

    with tile.TileContext(nc) as tc:
        with tc.tile_pool(name="data", bufs=1) as data, \
             tc.tile_pool(name="small", bufs=1) as small, \
             tc.tile_pool(name="work", bufs=3) as work:

            # ---- persistent SBUF tensors ----
            x = [data.tile([128, T], F16, tag=f"x{i}", name=f"x{i}") for i in range(2)]
            xq = [data.tile([128, THALF], F16, tag=f"xq{i}", name=f"xq{i}") for i in range(2)]
            xqf = [data.tile([128, THALF], F32, tag=f"xqf{i}", name=f"xqf{i}") for i in range(2)]
            wt = [data.tile([128, 3 * C], F32, tag=f"wt{i}", name=f"wt{i}") for i in range(2)]
            wtf = [data.tile([128, 3 * C], F16, tag=f"wtf{i}", name=f"wtf{i}") for i in range(2)]
            pjt = [data.tile([128, C], F16, tag=f"pjt{i}", name=f"pjt{i}") for i in range(2)]
            q_sb = [data.tile([128, THALF], F16, tag=f"q{i}", name=f"q{i}") for i in range(2)]
            k_sb = [data.tile([128, T], F16, tag=f"k{i}", name=f"k{i}") for i in range(2)]
            # vT: per s-chunk, per head: 64 v-columns + 1 ones-column (+3 pad)
            vt = data.tile([128, T // 128, NH, 2 * CH], P_DT, tag="vt", name="vt")
            a_sb = [data.tile([128, THALF], F16, tag=f"a{i}", name=f"a{i}") for i in range(2)]
            vbias_bc = data.tile([128, C], F32, tag="vbias_bc", name="vbias_bc")

            gnw = [small.tile([128, 1], F32, tag=f"gnw{i}", name=f"gnw{i}") for i in range(2)]
            gnb = [small.tile([128, 1], F32, tag=f"gnb{i}", name=f"gnb{i}") for i in range(2)]
            pjb = [small.tile([128, 1], F32, tag=f"pjb{i}", name=f"pjb{i}") for i in range(2)]
            qkb = [small.tile([128, 1], F32, tag=f"qkb{o}", name=f"qkb{o}") for o in range(4)]
            gsel = small.tile([128, 2 * G], F32, tag="gsel", name="gsel")
            bsel = small.tile([G, C], F32, tag="bsel", name="bsel")
            vbrow = small.tile([1, C], F32, tag="vbrow", name="vbrow")

            warm = small.tile([1, 1], F32, tag="warm", name="warm")
            nc.vector.memset(warm[:], 1.0)
            nc.scalar.activation(warm[:], warm[:], mybir.ActivationFunctionType.Ln)
            for i in range(2):
                cs = slice(128 * i, 128 * (i + 1))
                for ch8 in range(8):
                    t8 = slice(512 * ch8, 512 * (ch8 + 1))
                    nc.sync.dma_start(x[i][:, t8], d_xb[cs, t8])
                for ch2 in range(2):
                    t2 = slice(1024 * ch2, 1024 * (ch2 + 1))
                    nc.sync.dma_start(xq[i][:, t2], d_xq[cs, t2])
                nc.sync.dma_start(wt[i][:], d_wT[cs, :])
                nc.sync.dma_start(pjt[i][:], d_pjT[cs, :])
                nc.sync.dma_start(gnw[i][:], d_gnw[cs, :])
                nc.sync.dma_start(gnb[i][:], d_gnb[cs, :])
                nc.sync.dma_start(pjb[i][:], d_pjb[cs, :])
            for o in range(4):
                nc.sync.dma_start(qkb[o][:], d_qkvb[128 * o:128 * (o + 1), :])
            nc.sync.dma_start(gsel[:], d_gsel[:, :])
            nc.sync.dma_start(bsel[:], d_bsel[:, :])
            nc.sync.dma_start(vbrow[:], d_vbrow[:, :])
            for i in range(2):
                csx = slice(128 * i, 128 * (i + 1))
                nc.sync.dma_start(xqf[i][:], d_xqf[csx, :])  # residual-only, not startup-critical

            # ================= GroupNorm stats -> per-channel A, B ==========
            with tc.tile_pool(name="psQK", bufs=2, space="PSUM") as psQK, \
                 tc.tile_pool(name="psAV", bufs=2, space="PSUM") as psAV:
                psS = psAV
                psM = psAV

                A = [small.tile([128, 1], F32, tag=f"A{i}", name=f"A{i}") for i in range(2)]
                Bs = [small.tile([128, 1], F32, tag=f"B{i}", name=f"B{i}") for i in range(2)]
                pcs = [small.tile([128, 2], F32, tag=f"pcs{i}", name=f"pcs{i}") for i in range(2)]
                for i in range(2):
                    stats = work.tile([128, 8, 6], F32, tag="bnstats", name="bnstats")
                    for j in range(8):
                        nc.vector.bn_stats(stats[:, j, :], x[i][:, 512 * j:512 * (j + 1)])
                    mv = work.tile([128, 2], F32, tag="bnmv", name="bnmv")
                    nc.vector.bn_aggr(mv[:], stats[:])
                    # pcs = (mean, E[x^2]) per channel
                    nc.vector.tensor_copy(pcs[i][:, 0:1], mv[:, 0:1])
                    nc.vector.tensor_tensor(pcs[i][:, 1:2], mv[:, 0:1], mv[:, 0:1],
                                            mybir.AluOpType.mult)
                    nc.vector.tensor_tensor(pcs[i][:, 1:2], pcs[i][:, 1:2], mv[:, 1:2],
                                            mybir.AluOpType.add)

                # group stats [G, 2] = (mean_g, E[x^2]_g)
                grp_ps = psS.tile([G, 2], F32, tag="av", name="s")
                nc.tensor.matmul(grp_ps[:], gsel[:, 0:G], pcs[0][:], start=True, stop=False)
                nc.tensor.matmul(grp_ps[:], gsel[:, G:2 * G], pcs[1][:], start=False, stop=True)

                grp_sb = small.tile([G, 2], F32, tag="grp_sb", name="grp_sb")
                nc.vector.tensor_copy(grp_sb[:], grp_ps[:])
                grp2 = small.tile([G, 2], F32, tag="grp2", name="grp2")  # (mean, rstd)
                var = small.tile([G, 1], F32, tag="var", name="var")
                epst = small.tile([G, 1], F32, tag="epst", name="epst")
                nc.vector.memset(epst[:], EPS)
                nc.vector.tensor_copy(grp2[:, 0:1], grp_sb[:, 0:1])
                nc.vector.tensor_tensor(var[:], grp_sb[:, 0:1], grp_sb[:, 0:1],
                                        mybir.AluOpType.mult)
                nc.vector.tensor_tensor(var[:], grp_sb[:, 1:2], var[:],
                                        mybir.AluOpType.subtract)
                nc.scalar.activation(var[:], var[:], mybir.ActivationFunctionType.Ln,
                                     bias=epst[:])
                nc.scalar.activation(grp2[:, 1:2], var[:],
                                     mybir.ActivationFunctionType.Exp, scale=-0.5)

                # broadcast to channels; A = rstd*gn_w, B = gn_b - mean*A
                for i in range(2):
                    mb_ps = psS.tile([128, 2], F32, tag="av", name="s")
                    nc.tensor.matmul(mb_ps[:], bsel[:, 128 * i:128 * (i + 1)], grp2[:],
                                     start=True, stop=True)
                    nc.vector.tensor_tensor(A[i][:], mb_ps[:, 1:2], gnw[i][:],
                                            mybir.AluOpType.mult)
                    nc.vector.tensor_tensor(Bs[i][:], mb_ps[:, 0:1], A[i][:],
                                            mybir.AluOpType.mult)
                    nc.vector.tensor_tensor(Bs[i][:], gnb[i][:], Bs[i][:],
                                            mybir.AluOpType.subtract)
                    # folded weights
                    nc.vector.tensor_scalar_mul(wtf[i][:], wt[i][:], A[i][:])

                # folded q/k biases: qkvb[o] + sum_c wT[c,o]*B[c]
                for o in range(4):
                    b_ps = psS.tile([128, 1], F32, tag="av", name="s")
                    nc.tensor.matmul(b_ps[:], wt[0][:, 128 * o:128 * (o + 1)], Bs[0][:],
                                     start=True, stop=False)
                    nc.tensor.matmul(b_ps[:], wt[1][:, 128 * o:128 * (o + 1)], Bs[1][:],
                                     start=False, stop=True)
                    nc.vector.tensor_tensor(qkb[o][:], qkb[o][:], b_ps[:],
                                            mybir.AluOpType.add)
                # folded v bias as a row [1, C], then broadcast to 128 partitions
                vb_ps = psS.tile([1, C], F32, tag="av", name="s")
                nc.tensor.matmul(vb_ps[:], Bs[0][:], wt[0][:, 2 * C:3 * C],
                                 start=True, stop=False)
                nc.tensor.matmul(vb_ps[:], Bs[1][:], wt[1][:, 2 * C:3 * C],
                                 start=False, stop=True)
                vb_row = small.tile([1, C], F32, tag="vb_row", name="vb_row")
                nc.vector.tensor_tensor(vb_row[:], vb_ps[:], vbrow[:],
                                        mybir.AluOpType.add)
                nc.gpsimd.partition_broadcast(vbias_bc[:], vb_row[:])

                # ================= q / k / vT projections ====================
                # Emission order: q(o0,t0) -> k(o0,*) -> vT(*) -> rest, so the
                # first attention unit (pair 0) can start as early as possible.
                def proj_ps():
                    return psM.tile([128, 512], F32, tag="av", name="mm")

                def qk_pair(dst, wofs, src_t, bias, o, t):
                    # two t-tiles through one 2-bank qk-slot slice: 4 mms + 1 epilogue
                    ts2 = slice(512 * t, 512 * (t + 2))
                    ps = psQK.tile([128, 3, 512], F32, tag="qk", name="kps")[:, 0:2, :]
                    for u in range(2):
                        ts = slice(512 * (t + u), 512 * (t + u + 1))
                        nc.tensor.matmul(ps[:, u, :],
                                         wtf[0][:, wofs + 128 * o:wofs + 128 * (o + 1)],
                                         src_t[0][:, ts], start=True, stop=False)
                        nc.tensor.matmul(ps[:, u, :],
                                         wtf[1][:, wofs + 128 * o:wofs + 128 * (o + 1)],
                                         src_t[1][:, ts], start=False, stop=True)
                    nc.vector.tensor_scalar(dst[o][:, ts2].rearrange("p (u f) -> p u f", u=2),
                                            ps[:], bias[:], SCALE,
                                            mybir.AluOpType.add, mybir.AluOpType.mult)

                def vt_pair(sc):
                    # chunks sc, sc+1 through one 'av' slot: 4 mms + 1 epilogue
                    ps = proj_ps().rearrange("p (u f) -> p u f", u=2)
                    for u in range(2):
                        ss = slice(128 * (sc + u), 128 * (sc + u + 1))
                        nc.tensor.matmul(ps[:, u, :], x[0][:, ss], wtf[0][:, 2 * C:3 * C],
                                         start=True, stop=False)
                        nc.tensor.matmul(ps[:, u, :], x[1][:, ss], wtf[1][:, 2 * C:3 * C],
                                         start=False, stop=True)
                    nc.vector.tensor_tensor(
                        vt[:, sc:sc + 2, :, 0:CH],
                        ps.rearrange("p u (h c) -> p u h c", h=NH),
                        bass.AP(tensor=vbias_bc[:].tensor, offset=vbias_bc[:].offset,
                                ap=[vbias_bc[:].ap[0], [0, 2], [CH, NH], [1, CH]]),
                        mybir.AluOpType.add)

                nc.gpsimd.memset(vt[:, :, :, CH:CH + 1], 1.0)  # ones column (denominator)
                nc.gpsimd.memset(vt[:, :, :, CH + 1:2 * CH], 0.0)  # zero pad to 128 for FWL
                # all projections run upfront (mid-attention the AV accumulators
                # hold both 'av' PSUM slots, freezing any deferred production);
                # k/q pairs flow through the qk banks, vt pairs through 'av'.
                qk_pair(q_sb, 0, xq, qkb[0], 0, 0)
                for t in range(0, T // 512, 2):
                    qk_pair(k_sb, C, x, qkb[2], 0, t)
                for sc in range(0, T // 128, 2):
                    vt_pair(sc)
                # remaining production rides the unit boundaries (PSUM slots
                # rotate there); each lump is ~2 pairs, hidden under the exp
                # pipeline's double buffering
                boundary_prod = {
                    0: [lambda: qk_pair(q_sb, 0, xq, qkb[0], 0, 2)],
                    1: [lambda: qk_pair(k_sb, C, x, qkb[3], 1, 0),
                        lambda: qk_pair(k_sb, C, x, qkb[3], 1, 2)],
                    2: [lambda: qk_pair(k_sb, C, x, qkb[3], 1, 4),
                        lambda: qk_pair(k_sb, C, x, qkb[3], 1, 6)],
                    3: [lambda: qk_pair(q_sb, 0, xq, qkb[1], 1, 0),
                        lambda: qk_pair(q_sb, 0, xq, qkb[1], 1, 2)],
                }

                # ================= attention + proj ==============================
                # Head pairs (2p, 2p+1) interleaved matmul-by-matmul (disjoint PE
                # row groups -> concurrent QK). AV emission runs one exp-group
                # behind QK so the in-order PE queue never stalls on the ACT.
                NSC = T // 128                      # 32 s-chunks
                seq = []
                for sc in range(NSC):
                    seq.append((0, sc))
                    seq.append((1, sc))
                groups = [seq[g:g + 3] for g in range(0, len(seq), 3)]
                with tc.tile_pool(name="pexp", bufs=4) as pexp, \
                     tc.tile_pool(name="nrm", bufs=4) as nrm, \
                     tc.tile_pool(name="outp", bufs=3) as outp:
                    def emit_av(accs, p, grp, pe):
                        for j, (hi, sc) in enumerate(grp):
                            h = 2 * p + hi
                            nc.tensor.matmul(accs[hi][:, :], vt[:, sc, h, :],
                                             pe[:, j, :],
                                             start=(sc == 0), stop=(sc == NSC - 1))

                    def proj_steps(tb, o):
                        tbs = slice(512 * tb, 512 * (tb + 1))
                        box = {}
                        def s1():
                            box["pr"] = psM.tile([128, 512], F32, tag="av", name="pr")
                            nc.tensor.matmul(box["pr"][:], pjt[0][:, 128 * o:128 * (o + 1)],
                                             a_sb[0][:, tbs], start=True, stop=False)
                        def s2():
                            nc.tensor.matmul(box["pr"][:], pjt[1][:, 128 * o:128 * (o + 1)],
                                             a_sb[1][:, tbs], start=False, stop=True)
                        def s3():
                            res = outp.tile([128, 512], F32, tag="res", name="res")
                            box["res"] = res
                            nc.vector.tensor_scalar(res[:], box["pr"][:], pjb[o][:], None,
                                                    mybir.AluOpType.add)
                            nc.vector.tensor_tensor(res[:], res[:],
                                                    xqf[o][:, tbs],
                                                    mybir.AluOpType.add)
                            nc.sync.dma_start(d_out[128 * o:128 * (o + 1), tbs], res[:])
                        return [s1, s2, s3]

                    def proj_tb(tb):
                        for o in range(2):
                            for s in proj_steps(tb, o):
                                s()

                    def normalize(p, tb, accs):
                        tbs = slice(512 * tb, 512 * (tb + 1))
                        for hi in range(2):
                            hp = slice(64 * hi, 64 * hi + 64)
                            acc = accs[hi]
                            den = nrm.tile([1, 512], F32, tag="den", name="den")
                            nc.vector.tensor_copy(den[:], acc[CH:CH + 1, :])
                            rec = nrm.tile([1, 512], F32, tag="rec", name="rec")
                            nc.vector.reciprocal_approx_fast(rec[:], den[:])
                            bc = nrm.tile([CH, 512], F32, tag="bc", name="bc")
                            nc.gpsimd.partition_broadcast(bc[:], rec[:])
                            nc.vector.tensor_tensor(a_sb[p][hp, tbs], acc[0:CH, :],
                                                    bc[:], mybir.AluOpType.mult)

                    pend = None        # (accs, p, grp, pe) AV one group behind QK
                    fin = None         # (p, tb, accs) awaiting normalize/proj
                    units = [(p, tb) for p in range(2) for tb in range(THALF // 512)]
                    for (p, tb) in units:
                        tbs = slice(512 * tb, 512 * (tb + 1))
                        qk_t = q_sb[p]
                        kk_t = k_sb[p]
                        accs = [psAV.tile([128, 512], F32, tag="av", name="av")
                                for _ in range(2)]
                        for gi, grp in enumerate(groups):
                            qkp = psQK.tile([128, 3, 512], F32, tag="qk", name="qk")
                            for j, (hi, sc) in enumerate(grp):
                                hp = slice(64 * hi, 64 * hi + 64)
                                nc.tensor.matmul(
                                    qkp[:, j, :],
                                    kk_t[hp, 128 * sc:128 * (sc + 1)],
                                    qk_t[hp, tbs],
                                    start=True, stop=True)
                            pe = pexp.tile([128, 3, 512], P_DT, tag="pe", name="pe")
                            glen = len(grp)
                            nc.scalar.activation(pe[:, 0:glen, :], qkp[:, 0:glen, :],
                                                 mybir.ActivationFunctionType.Exp)
                            if pend is not None:
                                emit_av(*pend)
                            pend = (accs, p, grp, pe)
                            if fin is not None:
                                # previous unit's epilogue, after this unit's QK
                                # stream has started feeding the ACT
                                fp_, ftb_, faccs_ = fin
                                normalize(fp_, ftb_, faccs_)
                                if fp_ == 1:
                                    proj_tb(ftb_)
                                fin = None
                        fin = (p, tb, accs)
                        for fn_ in boundary_prod.pop(4 * p + tb, []):
                            fn_()
                    emit_av(*pend)
                    normalize(*fin)
                    proj_tb(fin[1])

    nc.compile()
    return nc


def _host_consts():
    g1 = np.zeros((128, G), dtype=np.float32)
    g2 = np.zeros((128, G), dtype=np.float32)
    for c in range(128):
        g1[c, c // CPG] = 1.0 / CPG
        g2[c, G // 2 + c // CPG] = 1.0 / CPG
    gsel = np.concatenate([g1, g2], axis=1)          # [128, 2G]
    bsel = np.zeros((G, C), dtype=np.float32)
    for c in range(C):
        bsel[c // CPG, c] = 1.0
    return gsel, bsel


def kernel(x, gn_w, gn_b, qkv_w, qkv_b, proj_w, proj_b):
    global LAST_RESULTS
    if "nc" not in _CACHE:
        _CACHE["nc"] = _build_program()
    nc = _CACHE["nc"]

    x = np.ascontiguousarray(np.asarray(x, dtype=np.float32))
    xr = x.reshape(B, C, T)
    gsel, bsel = _host_consts()
    shared = {
        "wT": np.ascontiguousarray(np.asarray(qkv_w, np.float32).T),
        "qkvb": np.asarray(qkv_b, np.float32).reshape(3 * C, 1).copy(),
        "vbrow": np.asarray(qkv_b, np.float32)[2 * C:].reshape(1, C).copy(),
        "pjT": np.ascontiguousarray(np.asarray(proj_w, np.float32).T.astype(np.float16)),
        "pjb": np.asarray(proj_b, np.float32).reshape(C, 1).copy(),
        "gnw": np.asarray(gn_w, np.float32).reshape(C, 1).copy(),
        "gnb": np.asarray(gn_b, np.float32).reshape(C, 1).copy(),
        "gsel": gsel,
        "bsel": bsel,
    }
    in_maps = []
    for c in range(NCORES):
        b, hf = c // 2, c % 2
        m = dict(shared)
        m["xb"] = np.ascontiguousarray(xr[b].astype(np.float16))
        xqs = np.ascontiguousarray(xr[b][:, hf * THALF:(hf + 1) * THALF])
        m["xq"] = xqs.astype(np.float16)
        m["xqf"] = xqs
        in_maps.append(m)

    res = bass_utils.run_bass_kernel_spmd(nc, in_maps, core_ids=list(range(NCORES)))
    LAST_RESULTS = res

    out = np.empty((B, C, T), dtype=np.float32)
    for c in range(NCORES):
        b, hf = c // 2, c % 2
        out[b][:, hf * THALF:(hf + 1) * THALF] = res.results[c]["out"]
    return out.reshape(B, C, HH, WW)



# revision 17
# speedup vs baseline: 1.1510x; 1.1510x over previous
"""AttentionBlock (GroupNorm + 1x1 QKV + MHA + proj + residual) on 8 trn2 cores.

Sharding: core c -> (batch b = c//2, t-half = c%2). Each core computes all 4
heads for its 2048 query positions; k/v are computed over the full T=4096 from
the core's batch. No cross-core communication needed.

v2 changes (vs the 363us v1):
- The softmax exp (33.5M elems/core -- the v1 critical path, ~318us of
  ScalarE ACTIVATE) is split across THREE engines: ScalarE does exact exp,
  VectorE and GpSimd run a Schraudolph bits-trick exp (one tensor_scalar
  f32->int8 writing e5m2 exponent bits). Softmax normalization cancels the
  approximation's systematic error; measured end-to-end rel err ~1.5e-3.
- Attention weights (pe) are fp8 e5m2 and v is fp8 e4m3; the AV matmul runs
  in DoubleRow perf mode (256-row contraction per MM) with the softmax
  denominator ones-column as a 65th lhsT column -> 16 AV matmuls per unit
  instead of 64.
- QK keeps f16 operands with 2-head PE row-group packing (concurrent MMs).
- Scores are computed TRANSPOSED, S_T[s, t] (s on partitions), so AV needs
  no transposes; GroupNorm is folded into the QKV weights.
"""

import math
from collections import deque

import numpy as np

import concourse.bass as bass
import concourse.tile as tile
from concourse import bacc, mybir
from concourse import bass_utils

F32 = mybir.dt.float32
BF16 = mybir.dt.bfloat16
F16 = mybir.dt.float16
F8E5 = mybir.dt.float8e5
F8E4 = mybir.dt.float8e4
I8 = mybir.dt.int8
I16 = mybir.dt.int16

B, C, HH, WW = 4, 256, 64, 64
T = HH * WW            # 4096
NH = 4                 # heads per batch
CH = C // NH           # 64 channels per head
G = 32                 # groupnorm groups
CPG = C // G           # 8 channels per group
EPS = 1e-5
NCORES = 8
THALF = T // 2         # 2048  t-columns per core
SCALE = 1.0 / math.sqrt(math.sqrt(CH))
NSC = T // 128         # 32 s-chunks
NSCP = NSC // 2        # 16 s-chunk pairs (DoubleRow)

# Schraudolph exp -> e5m2 bits: bits8 = round(l * 4*log2(e) + B8)
A8 = 4.0 / math.log(2.0)
B8 = 59.75

# engine split per unit: 32 exp calls (2 items each). GPSIMD cannot read
# PSUM (walrus birverifier), so only ScalarE + VectorE exp the scores;
# GpSimd gets the SBUF-side work (broadcasts, residual adds).
ENG_COUNTS = {"A": 17, "D": 15}

_CACHE = {}
LAST_RESULTS = None


def _eng_pattern(counts=None):
    counts = counts or ENG_COUNTS
    n = sum(counts.values())
    acc = {k: 0.0 for k in counts}
    seq = []
    for _ in range(n):
        for k in counts:
            acc[k] += counts[k] / n
        k = max(acc, key=lambda x: acc[x])
        acc[k] -= 1.0
        seq.append(k)
    return seq


def _build_program():
    nc = bacc.Bacc("TRN2", target_bir_lowering=False, debug=False)

    d_xb = nc.dram_tensor("xb", [C, T], F16, kind="ExternalInput").ap()
    d_xq = nc.dram_tensor("xq", [C, THALF], F16, kind="ExternalInput").ap()
    d_xqf = nc.dram_tensor("xqf", [C, THALF], F32, kind="ExternalInput").ap()
    d_wT = nc.dram_tensor("wT", [C, 3 * C], F32, kind="ExternalInput").ap()
    d_qkvb = nc.dram_tensor("qkvb", [3 * C, 1], F32, kind="ExternalInput").ap()
    d_vbrow = nc.dram_tensor("vbrow", [1, C], F32, kind="ExternalInput").ap()
    d_pjT = nc.dram_tensor("pjT", [C, C], F16, kind="ExternalInput").ap()
    d_pjb = nc.dram_tensor("pjb", [C, 1], F32, kind="ExternalInput").ap()
    d_gnw = nc.dram_tensor("gnw", [C, 1], F32, kind="ExternalInput").ap()
    d_gnb = nc.dram_tensor("gnb", [C, 1], F32, kind="ExternalInput").ap()
    d_gsel = nc.dram_tensor("gsel", [128, 2 * G], F32, kind="ExternalInput").ap()
    d_bsel = nc.dram_tensor("bsel", [G, C], F32, kind="ExternalInput").ap()
    d_out = nc.dram_tensor("out", [C, THALF], F32, kind="ExternalOutput").ap()

    with tile.TileContext(nc) as tc:
        with tc.tile_pool(name="data", bufs=1) as data, \
             tc.tile_pool(name="small", bufs=1) as small, \
             tc.tile_pool(name="work", bufs=3) as work:

            # ---- persistent SBUF tensors ----
            x = [data.tile([128, T], F16, tag=f"x{i}", name=f"x{i}") for i in range(2)]
            xq = [data.tile([128, THALF], F16, tag=f"xq{i}", name=f"xq{i}") for i in range(2)]
            xqf = [data.tile([128, THALF], F32, tag=f"xqf{i}", name=f"xqf{i}") for i in range(2)]
            wt = [data.tile([128, 3 * C], F32, tag=f"wt{i}", name=f"wt{i}") for i in range(2)]
            wtf = [data.tile([128, 3 * C], F16, tag=f"wtf{i}", name=f"wtf{i}") for i in range(2)]
            pjt = [data.tile([128, C], F16, tag=f"pjt{i}", name=f"pjt{i}") for i in range(2)]
            q_sb = [data.tile([128, THALF], F16, tag=f"q{i}", name=f"q{i}") for i in range(2)]
            k_sb = [data.tile([128, T], F16, tag=f"k{i}", name=f"k{i}") for i in range(2)]
            # vT for DoubleRow AV: [s-in-chunk, scp, chunk-parity, head, col]
            # col 0..63 = v channels, col 64 = ones (denominator), 65..79 pad
            # (80B stride keeps the DoubleRow middle-dim step 16B-aligned).
            vt = data.tile([128, NSCP, 2, NH, 80], F8E4, tag="vt", name="vt")
            a_sb = [data.tile([128, THALF], F16, tag=f"a{i}", name=f"a{i}") for i in range(2)]
            vbias_bc = data.tile([128, C], F32, tag="vbias_bc", name="vbias_bc")

            gnw = [small.tile([128, 1], F32, tag=f"gnw{i}", name=f"gnw{i}") for i in range(2)]
            gnb = [small.tile([128, 1], F32, tag=f"gnb{i}", name=f"gnb{i}") for i in range(2)]
            pjb = [small.tile([128, 1], F32, tag=f"pjb{i}", name=f"pjb{i}") for i in range(2)]
            qkb = [small.tile([128, 1], F32, tag=f"qkb{o}", name=f"qkb{o}") for o in range(4)]
            gsel = small.tile([128, 2 * G], F32, tag="gsel", name="gsel")
            bsel = small.tile([G, C], F32, tag="bsel", name="bsel")
            vbrow = small.tile([1, C], F32, tag="vbrow", name="vbrow")

            warm = small.tile([1, 1], F32, tag="warm", name="warm")
            nc.vector.memset(warm[:], 1.0)
            nc.scalar.activation(warm[:], warm[:], mybir.ActivationFunctionType.Ln)
            b8t = small.tile([128, 1], F32, tag="b8t", name="b8t")
            nc.vector.memset(b8t[:], B8)
            for i in range(2):
                cs = slice(128 * i, 128 * (i + 1))
                for ch8 in range(8):
                    t8 = slice(512 * ch8, 512 * (ch8 + 1))
                    nc.sync.dma_start(x[i][:, t8], d_xb[cs, t8])
                for ch2 in range(2):
                    t2 = slice(1024 * ch2, 1024 * (ch2 + 1))
                    nc.sync.dma_start(xq[i][:, t2], d_xq[cs, t2])
                nc.sync.dma_start(wt[i][:], d_wT[cs, :])
                nc.sync.dma_start(pjt[i][:], d_pjT[cs, :])
                nc.sync.dma_start(gnw[i][:], d_gnw[cs, :])
                nc.sync.dma_start(gnb[i][:], d_gnb[cs, :])
                nc.sync.dma_start(pjb[i][:], d_pjb[cs, :])
            for o in range(4):
                nc.sync.dma_start(qkb[o][:], d_qkvb[128 * o:128 * (o + 1), :])
            nc.sync.dma_start(gsel[:], d_gsel[:, :])
            nc.sync.dma_start(bsel[:], d_bsel[:, :])
            nc.sync.dma_start(vbrow[:], d_vbrow[:, :])
            for i in range(2):
                csx = slice(128 * i, 128 * (i + 1))
                nc.sync.dma_start(xqf[i][:], d_xqf[csx, :])  # residual-only, not startup-critical

            # ones column for the softmax denominator (written once; the v
            # projection only writes cols 0..63, pad cols 65..79 never read)
            nc.gpsimd.memset(vt[:, :, :, :, 64:65], 1.0)

            # PSUM: 3x qk slots (2 banks each) + 2x av accumulators (1 bank each)
            with tc.tile_pool(name="psQK", bufs=3, space="PSUM") as psQK, \
                 tc.tile_pool(name="psACC", bufs=2, space="PSUM") as psACC:

                # ================= GroupNorm stats -> per-channel A, B ==========
                A = [small.tile([128, 1], F32, tag=f"A{i}", name=f"A{i}") for i in range(2)]
                Bs = [small.tile([128, 1], F32, tag=f"B{i}", name=f"B{i}") for i in range(2)]
                pcs = [small.tile([128, 2], F32, tag=f"pcs{i}", name=f"pcs{i}") for i in range(2)]
                for i in range(2):
                    stats = work.tile([128, 8, 6], F32, tag="bnstats", name="bnstats")
                    for j in range(8):
                        nc.vector.bn_stats(stats[:, j, :], x[i][:, 512 * j:512 * (j + 1)])
                    mv = work.tile([128, 2], F32, tag="bnmv", name="bnmv")
                    nc.vector.bn_aggr(mv[:], stats[:])
                    # pcs = (mean, E[x^2]) per channel
                    nc.vector.tensor_copy(pcs[i][:, 0:1], mv[:, 0:1])
                    nc.vector.tensor_tensor(pcs[i][:, 1:2], mv[:, 0:1], mv[:, 0:1],
                                            mybir.AluOpType.mult)
                    nc.vector.tensor_tensor(pcs[i][:, 1:2], pcs[i][:, 1:2], mv[:, 1:2],
                                            mybir.AluOpType.add)

                # group stats [G, 2] = (mean_g, E[x^2]_g)
                grp_ps = psACC.tile([G, 2], F32, tag="av", name="s")
                nc.tensor.matmul(grp_ps[:], gsel[:, 0:G], pcs[0][:], start=True, stop=False)
                nc.tensor.matmul(grp_ps[:], gsel[:, G:2 * G], pcs[1][:], start=False, stop=True)

                grp_sb = small.tile([G, 2], F32, tag="grp_sb", name="grp_sb")
                nc.vector.tensor_copy(grp_sb[:], grp_ps[:])
                grp2 = small.tile([G, 2], F32, tag="grp2", name="grp2")  # (mean, rstd)
                var = small.tile([G, 1], F32, tag="var", name="var")
                epst = small.tile([G, 1], F32, tag="epst", name="epst")
                nc.vector.memset(epst[:], EPS)
                nc.vector.tensor_copy(grp2[:, 0:1], grp_sb[:, 0:1])
                nc.vector.tensor_tensor(var[:], grp_sb[:, 0:1], grp_sb[:, 0:1],
                                        mybir.AluOpType.mult)
                nc.vector.tensor_tensor(var[:], grp_sb[:, 1:2], var[:],
                                        mybir.AluOpType.subtract)
                nc.scalar.activation(var[:], var[:], mybir.ActivationFunctionType.Ln,
                                     bias=epst[:])
                nc.scalar.activation(grp2[:, 1:2], var[:],
                                     mybir.ActivationFunctionType.Exp, scale=-0.5)

                # broadcast to channels; A = rstd*gn_w, B = gn_b - mean*A
                for i in range(2):
                    mb_ps = psACC.tile([128, 2], F32, tag="av", name="s")
                    nc.tensor.matmul(mb_ps[:], bsel[:, 128 * i:128 * (i + 1)], grp2[:],
                                     start=True, stop=True)
                    nc.vector.tensor_tensor(A[i][:], mb_ps[:, 1:2], gnw[i][:],
                                            mybir.AluOpType.mult)
                    nc.vector.tensor_tensor(Bs[i][:], mb_ps[:, 0:1], A[i][:],
                                            mybir.AluOpType.mult)
                    nc.vector.tensor_tensor(Bs[i][:], gnb[i][:], Bs[i][:],
                                            mybir.AluOpType.subtract)
                    # folded weights
                    nc.vector.tensor_scalar_mul(wtf[i][:], wt[i][:], A[i][:])

                # folded q/k biases: qkvb[o] + sum_c wT[c,o]*B[c]
                for o in range(4):
                    b_ps = psACC.tile([128, 1], F32, tag="av", name="s")
                    nc.tensor.matmul(b_ps[:], wt[0][:, 128 * o:128 * (o + 1)], Bs[0][:],
                                     start=True, stop=False)
                    nc.tensor.matmul(b_ps[:], wt[1][:, 128 * o:128 * (o + 1)], Bs[1][:],
                                     start=False, stop=True)
                    nc.vector.tensor_tensor(qkb[o][:], qkb[o][:], b_ps[:],
                                            mybir.AluOpType.add)
                # folded v bias as a row [1, C], then broadcast to 128 partitions
                vb_ps = psACC.tile([1, C], F32, tag="av", name="s")
                nc.tensor.matmul(vb_ps[:], Bs[0][:], wt[0][:, 2 * C:3 * C],
                                 start=True, stop=False)
                nc.tensor.matmul(vb_ps[:], Bs[1][:], wt[1][:, 2 * C:3 * C],
                                 start=False, stop=True)
                vb_row = small.tile([1, C], F32, tag="vb_row", name="vb_row")
                nc.vector.tensor_tensor(vb_row[:], vb_ps[:], vbrow[:],
                                        mybir.AluOpType.add)
                nc.gpsimd.partition_broadcast(vbias_bc[:], vb_row[:])

                # scaled q/k biases for the ScalarE epilogue path:
                # (x + b)*SCALE == Copy(SCALE*x + SCALE*b)
                qkbS = [small.tile([128, 1], F32, tag=f"qkbS{o}", name=f"qkbS{o}")
                        for o in range(4)]
                for o in range(4):
                    nc.vector.tensor_scalar_mul(qkbS[o][:], qkb[o][:], SCALE)

                # ================= q / k / vT production =====================
                def qk_pair(dst, wofs, src_t, bias_i, o, t, eng="D"):
                    # two t-tiles through one qk slot: 4 mms + 1 epilogue
                    ts2 = slice(512 * t, 512 * (t + 2))
                    ps = psQK.tile([128, 2, 512], F32, tag="qk", name="kps")
                    for u in range(2):
                        ts = slice(512 * (t + u), 512 * (t + u + 1))
                        nc.tensor.matmul(ps[:, u, :],
                                         wtf[0][:, wofs + 128 * o:wofs + 128 * (o + 1)],
                                         src_t[0][:, ts], start=True, stop=False)
                        nc.tensor.matmul(ps[:, u, :],
                                         wtf[1][:, wofs + 128 * o:wofs + 128 * (o + 1)],
                                         src_t[1][:, ts], start=False, stop=True)
                    dst_ap = dst[o][:, ts2].rearrange("p (u f) -> p u f", u=2)
                    if eng == "A":
                        nc.scalar.activation(dst_ap, ps[:],
                                             mybir.ActivationFunctionType.Identity,
                                             bias=qkbS[bias_i][:], scale=SCALE)
                    else:
                        nc.vector.tensor_scalar(dst_ap, ps[:], qkb[bias_i][:], SCALE,
                                                mybir.AluOpType.add,
                                                mybir.AluOpType.mult)

                def vt_pair(j):
                    # s-chunks 2j, 2j+1 -> vt[:, j, 0:2, :, 0:64]
                    ps = psQK.tile([128, 2, 512], F32, tag="qk", name="vps")[:, :, 0:256]
                    for u in range(2):
                        ss = slice(128 * (2 * j + u), 128 * (2 * j + u + 1))
                        nc.tensor.matmul(ps[:, u, :], x[0][:, ss], wtf[0][:, 2 * C:3 * C],
                                         start=True, stop=False)
                        nc.tensor.matmul(ps[:, u, :], x[1][:, ss], wtf[1][:, 2 * C:3 * C],
                                         start=False, stop=True)
                    nc.vector.tensor_tensor(
                        vt[:, j, :, :, 0:CH],
                        ps.rearrange("p u (h c) -> p u h c", h=NH),
                        bass.AP(tensor=vbias_bc[:].tensor, offset=vbias_bc[:].offset,
                                ap=[vbias_bc[:].ap[0], [0, 2], [CH, NH], [1, CH]]),
                        mybir.AluOpType.add)

                # minimal pre-work for unit (0,0): k heads 0-1, q heads 0-1
                # t 0..1023, first 4 vt chunk-pairs. The rest rides the units.
                # PRE epilogues go to ScalarE (it idles during startup).
                qk_pair(q_sb, 0, xq, 0, 0, 0, "A")
                for t in range(0, 8, 2):
                    qk_pair(k_sb, C, x, 2, 0, t, "A")
                for j in range(4):
                    vt_pair(j)

                # per-unit production hooks: {(p, tb): {scp: [fn, ...]}}
                hooks = {
                    (0, 0): {j - 3: [lambda j=j: vt_pair(j)] for j in range(4, NSCP)},
                    (0, 1): {3: [lambda: qk_pair(k_sb, C, x, 3, 1, 0, "A")],
                             8: [lambda: qk_pair(k_sb, C, x, 3, 1, 2, "D")],
                             12: [lambda: qk_pair(q_sb, 0, xq, 0, 0, 2, "A")]},
                    (0, 2): {3: [lambda: qk_pair(k_sb, C, x, 3, 1, 4, "D")],
                             8: [lambda: qk_pair(k_sb, C, x, 3, 1, 6, "A")],
                             12: [lambda: qk_pair(q_sb, 0, xq, 1, 1, 0, "D")]},
                    (0, 3): {8: [lambda: qk_pair(q_sb, 0, xq, 1, 1, 2, "A")]},
                }

                # ================= attention + proj ==========================
                with tc.tile_pool(name="pexp", bufs=8) as pexp, \
                     tc.tile_pool(name="nrm", bufs=4) as nrm, \
                     tc.tile_pool(name="outp", bufs=3) as outp:

                    def emit_exp(eng, pe, slot):
                        if eng == "A":
                            # same Schraudolph bits on ScalarE: walrus rejects
                            # fp8 ACT output, so write the e5m2 bits via int8.
                            # Relu clamps the negative-bits hazard to +0.0.
                            nc.scalar.activation(pe[:].bitcast(I8), slot[:],
                                                 mybir.ActivationFunctionType.Relu,
                                                 bias=b8t[:], scale=A8)
                        else:
                            nc.vector.tensor_scalar(pe[:].bitcast(I8), slot[:], A8, B8,
                                                    mybir.AluOpType.mult,
                                                    mybir.AluOpType.add)

                    def emit_avden(p, accs, hi, scp, pe):
                        h = 2 * p + hi
                        nc.tensor.matmul(accs[hi][0:65, :],
                                         vt[:, scp, :, h, 0:65],
                                         pe[:],
                                         start=(scp == 0), stop=(scp == NSCP - 1),
                                         perf_mode=mybir.MatmulPerfMode.DoubleRow)

                    def normalize(p, tb, accs):
                        tbs = slice(512 * tb, 512 * (tb + 1))
                        for hi in range(2):
                            hp = slice(64 * hi, 64 * hi + 64)
                            acc = accs[hi]
                            den = nrm.tile([1, 512], F32, tag="den", name="den")
                            nc.vector.tensor_copy(den[:], acc[64:65, :])
                            rec = nrm.tile([1, 512], F32, tag="rec", name="rec")
                            nc.vector.reciprocal_approx_fast(rec[:], den[:])
                            bc = nrm.tile([CH, 512], F32, tag="bc", name="bc")
                            nc.gpsimd.partition_broadcast(bc[:], rec[:])
                            # acc is PSUM -> GpSimd can't read it; DVE only
                            nc.vector.tensor_tensor(a_sb[p][hp, tbs], acc[0:CH, :],
                                                    bc[:], mybir.AluOpType.mult)

                    def proj_tb(tb):
                        tbs = slice(512 * tb, 512 * (tb + 1))
                        for o in range(2):
                            pr = psQK.tile([128, 512], F32, tag="qk", name="pr")
                            nc.tensor.matmul(pr[:], pjt[0][:, 128 * o:128 * (o + 1)],
                                             a_sb[0][:, tbs], start=True, stop=False)
                            nc.tensor.matmul(pr[:], pjt[1][:, 128 * o:128 * (o + 1)],
                                             a_sb[1][:, tbs], start=False, stop=True)
                            # PSUM+bias on ScalarE, then the SBUF-only residual
                            # add on GpSimd -- keeps VectorE free for exp
                            res = outp.tile([128, 512], F32, tag="res", name="res")
                            nc.scalar.activation(res[:], pr[:],
                                                 mybir.ActivationFunctionType.Identity,
                                                 bias=pjb[o][:])
                            nc.gpsimd.tensor_tensor(res[:], res[:], xqf[o][:, tbs],
                                                    mybir.AluOpType.add)
                            nc.sync.dma_start(d_out[128 * o:128 * (o + 1), tbs], res[:])

                    pattern_std = _eng_pattern()
                    # unit (0,0) carries 12 vt epilogues on VectorE -> shift
                    # its exp calls toward ScalarE
                    pattern_u0 = _eng_pattern({"A": 22, "D": 10})
                    units = [(p, tb) for p in range(2) for tb in range(THALF // 512)]
                    fin = None          # (p, tb, accs) awaiting normalize/proj
                    for (p, tb) in units:
                        pattern = pattern_u0 if (p, tb) == (0, 0) else pattern_std
                        tbs = slice(512 * tb, 512 * (tb + 1))
                        uhooks = hooks.get((p, tb), {})
                        accs = [psACC.tile([128, 512], F32, tag="av", name="av")
                                for _ in range(2)]
                        pend = deque()
                        ci = 0
                        for scp in range(NSCP):
                            for hi in range(2):
                                slot = psQK.tile([128, 2, 512], F32, tag="qk", name="qk")
                                hp = slice(64 * hi, 64 * hi + 64)
                                for u in range(2):
                                    sc = 2 * scp + u
                                    nc.tensor.matmul(
                                        slot[:, u, :],
                                        k_sb[p][hp, 128 * sc:128 * (sc + 1)],
                                        q_sb[p][hp, tbs],
                                        start=True, stop=True)
                                pe = pexp.tile([128, 2, 512], F8E5, tag="pe", name="pe")
                                emit_exp(pattern[ci], pe, slot)
                                ci += 1
                                pend.append((hi, scp, pe))
                                while len(pend) > 3:
                                    emit_avden(p, accs, *pend.popleft())
                            if scp == 0 and fin is not None:
                                # previous unit's epilogue, after this unit's QK
                                # stream has started feeding the engines
                                normalize(fin[0], fin[1], fin[2])
                            if scp == 1 and fin is not None:
                                if fin[0] == 1:
                                    proj_tb(fin[1])
                                fin = None
                            for fn_ in uhooks.get(scp, []):
                                fn_()
                        while pend:
                            emit_avden(p, accs, *pend.popleft())
                        fin = (p, tb, accs)
                    normalize(*fin)
                    proj_tb(fin[1])

    nc.compile()
    return nc


def _host_consts():
    g1 = np.zeros((128, G), dtype=np.float32)
    g2 = np.zeros((128, G), dtype=np.float32)
    for c in range(128):
        g1[c, c // CPG] = 1.0 / CPG
        g2[c, G // 2 + c // CPG] = 1.0 / CPG
    gsel = np.concatenate([g1, g2], axis=1)          # [128, 2G]
    bsel = np.zeros((G, C), dtype=np.float32)
    for c in range(C):
        bsel[c // CPG, c] = 1.0
    return gsel, bsel


def kernel(x, gn_w, gn_b, qkv_w, qkv_b, proj_w, proj_b):
    global LAST_RESULTS
    if "nc" not in _CACHE:
        _CACHE["nc"] = _build_program()
    nc = _CACHE["nc"]

    x = np.ascontiguousarray(np.asarray(x, dtype=np.float32))
    xr = x.reshape(B, C, T)
    gsel, bsel = _host_consts()
    shared = {
        "wT": np.ascontiguousarray(np.asarray(qkv_w, np.float32).T),
        "qkvb": np.asarray(qkv_b, np.float32).reshape(3 * C, 1).copy(),
        "vbrow": np.asarray(qkv_b, np.float32)[2 * C:].reshape(1, C).copy(),
        "pjT": np.ascontiguousarray(np.asarray(proj_w, np.float32).T.astype(np.float16)),
        "pjb": np.asarray(proj_b, np.float32).reshape(C, 1).copy(),
        "gnw": np.asarray(gn_w, np.float32).reshape(C, 1).copy(),
        "gnb": np.asarray(gn_b, np.float32).reshape(C, 1).copy(),
        "gsel": gsel,
        "bsel": bsel,
    }
    in_maps = []
    for c in range(NCORES):
        b, hf = c // 2, c % 2
        m = dict(shared)
        m["xb"] = np.ascontiguousarray(xr[b].astype(np.float16))
        xqs = np.ascontiguousarray(xr[b][:, hf * THALF:(hf + 1) * THALF])
        m["xq"] = xqs.astype(np.float16)
        m["xqf"] = xqs
        in_maps.append(m)

    res = bass_utils.run_bass_kernel_spmd(nc, in_maps, core_ids=list(range(NCORES)))
    LAST_RESULTS = res

    out = np.empty((B, C, T), dtype=np.float32)
    for c in range(NCORES):
        b, hf = c // 2, c % 2
        out[b][:, hf * THALF:(hf + 1) * THALF] = res.results[c]["out"]
    return out.reshape(B, C, HH, WW)


# revision 27
# speedup vs baseline: 1.2593x; 1.0941x over previous
"""AttentionBlock (GroupNorm + 1x1 QKV + MHA + proj + residual) on 8 trn2 cores.

Sharding: core c -> (batch b = c//2, t-half = c%2). Each core computes all 4
heads for its 2048 query positions; k/v are computed over the full T=4096 from
the core's batch. No cross-core communication needed.

v2 changes (vs the 363us v1):
- The softmax exp (33.5M elems/core -- the v1 critical path, ~318us of
  ScalarE ACTIVATE) is split across THREE engines: ScalarE does exact exp,
  VectorE and GpSimd run a Schraudolph bits-trick exp (one tensor_scalar
  f32->int8 writing e5m2 exponent bits). Softmax normalization cancels the
  approximation's systematic error; measured end-to-end rel err ~1.5e-3.
- Attention weights (pe) are fp8 e5m2 and v is fp8 e4m3; the AV matmul runs
  in DoubleRow perf mode (256-row contraction per MM) with the softmax
  denominator ones-column as a 65th lhsT column -> 16 AV matmuls per unit
  instead of 64.
- QK keeps f16 operands with 2-head PE row-group packing (concurrent MMs).
- Scores are computed TRANSPOSED, S_T[s, t] (s on partitions), so AV needs
  no transposes; GroupNorm is folded into the QKV weights.
"""

import math
from collections import deque

import numpy as np

import concourse.bass as bass
import concourse.tile as tile
from concourse import bacc, mybir
from concourse import bass_utils

F32 = mybir.dt.float32
BF16 = mybir.dt.bfloat16
F16 = mybir.dt.float16
F8E5 = mybir.dt.float8e5
F8E4 = mybir.dt.float8e4
I8 = mybir.dt.int8
I16 = mybir.dt.int16

B, C, HH, WW = 4, 256, 64, 64
T = HH * WW            # 4096
NH = 4                 # heads per batch
CH = C // NH           # 64 channels per head
G = 32                 # groupnorm groups
CPG = C // G           # 8 channels per group
EPS = 1e-5
NCORES = 8
THALF = T // 2         # 2048  t-columns per core
SCALE = 1.0 / math.sqrt(math.sqrt(CH))
NSC = T // 128         # 32 s-chunks
NSCP = NSC // 2        # 16 s-chunk pairs (DoubleRow)

# Schraudolph exp -> e5m2 bits: bits8 = round(l * 4*log2(e) + B8)
A8 = 4.0 / math.log(2.0)
B8 = 59.75

# engine split per unit: 32 exp calls (2 items each). GPSIMD cannot read
# PSUM (walrus birverifier), so only ScalarE + VectorE exp the scores;
# GpSimd gets the SBUF-side work (broadcasts, residual adds).
ENG_COUNTS = {"A": 17, "D": 15}

_CACHE = {}
LAST_RESULTS = None


def _eng_pattern(counts=None):
    counts = counts or ENG_COUNTS
    n = sum(counts.values())
    acc = {k: 0.0 for k in counts}
    seq = []
    for _ in range(n):
        for k in counts:
            acc[k] += counts[k] / n
        k = max(acc, key=lambda x: acc[x])
        acc[k] -= 1.0
        seq.append(k)
    return seq


def _build_program():
    nc = bacc.Bacc("TRN2", target_bir_lowering=False, debug=False)

    d_xb = nc.dram_tensor("xb", [C, T], F16, kind="ExternalInput").ap()
    d_xq = nc.dram_tensor("xq", [C, THALF], F16, kind="ExternalInput").ap()
    d_xqf = nc.dram_tensor("xqf", [C, THALF], F32, kind="ExternalInput").ap()
    d_wT = nc.dram_tensor("wT", [C, 3 * C], F32, kind="ExternalInput").ap()
    # host-packed small params: cols 0:2 gnw, 2:4 gnb, 4:6 pjb, 6:12 qkvb
    # (6 col-blocks of 128), 12:76 gsel — one DMA instead of 13
    d_smalls = nc.dram_tensor("smalls", [128, 76], F32, kind="ExternalInput").ap()
    d_vbrow = nc.dram_tensor("vbrow", [1, C], F32, kind="ExternalInput").ap()
    d_pjT = nc.dram_tensor("pjT", [C, C], F16, kind="ExternalInput").ap()
    d_bsel = nc.dram_tensor("bsel", [G, C], F32, kind="ExternalInput").ap()
    d_out = nc.dram_tensor("out", [C, THALF], F32, kind="ExternalOutput").ap()

    with tile.TileContext(nc) as tc:
        with tc.tile_pool(name="data", bufs=1) as data, \
             tc.tile_pool(name="small", bufs=1) as small, \
             tc.tile_pool(name="work", bufs=3) as work:

            # ---- persistent SBUF tensors ----
            x = [data.tile([128, T], F16, tag=f"x{i}", name=f"x{i}") for i in range(2)]
            xq = [data.tile([128, THALF], F16, tag=f"xq{i}", name=f"xq{i}") for i in range(2)]
            xqf = [data.tile([128, THALF], F32, tag=f"xqf{i}", name=f"xqf{i}") for i in range(2)]
            wt = [data.tile([128, 3 * C], F32, tag=f"wt{i}", name=f"wt{i}") for i in range(2)]
            wtf = [data.tile([128, 3 * C], F16, tag=f"wtf{i}", name=f"wtf{i}") for i in range(2)]
            pjt = [data.tile([128, C], F16, tag=f"pjt{i}", name=f"pjt{i}") for i in range(2)]
            q_sb = [data.tile([128, THALF], F16, tag=f"q{i}", name=f"q{i}") for i in range(2)]
            k_sb = [data.tile([128, T], F16, tag=f"k{i}", name=f"k{i}") for i in range(2)]
            # vT for DoubleRow AV: [s-in-chunk, scp, chunk-parity, head, col]
            # col 0..63 = v channels, col 64 = ones (denominator), 65..79 pad
            # (80B stride keeps the DoubleRow middle-dim step 16B-aligned).
            vt = data.tile([128, NSCP, 2, NH, 80], F8E4, tag="vt", name="vt")
            a_sb = [data.tile([128, THALF], F16, tag=f"a{i}", name=f"a{i}") for i in range(2)]
            vbias_bc = data.tile([128, C], F32, tag="vbias_bc", name="vbias_bc")

            smalls = small.tile([128, 76], F32, tag="smalls", name="smalls")
            gnw = [smalls[:, i:i + 1] for i in range(2)]
            gnb = [smalls[:, 2 + i:3 + i] for i in range(2)]
            pjb = [smalls[:, 4 + i:5 + i] for i in range(2)]
            qkb4 = small.tile([128, 4], F32, tag="qkb4", name="qkb4")
            qkb = [qkb4[:, o:o + 1] for o in range(4)]
            gsel = smalls[:, 12:76]
            bsel = small.tile([G, C], F32, tag="bsel", name="bsel")
            vbrow = small.tile([1, C], F32, tag="vbrow", name="vbrow")

            warm = small.tile([1, 1], F32, tag="warm", name="warm")
            nc.vector.memset(warm[:], 1.0)
            nc.scalar.activation(warm[:], warm[:], mybir.ActivationFunctionType.Ln)
            b8t = small.tile([128, 1], F32, tag="b8t", name="b8t")
            nc.vector.memset(b8t[:], B8)
            # x first (GN stats gate everything), halves interleaved so both
            # stats pipelines start early; xqf (residual-only) last
            for ch4 in range(4):
                t4 = slice(1024 * ch4, 1024 * (ch4 + 1))
                for i in range(2):
                    cs = slice(128 * i, 128 * (i + 1))
                    nc.sync.dma_start(x[i][:, t4], d_xb[cs, t4])
            for i in range(2):
                cs = slice(128 * i, 128 * (i + 1))
                nc.sync.dma_start(wt[i][:], d_wT[cs, :])
            nc.sync.dma_start(smalls[:], d_smalls[:, :])
            nc.sync.dma_start(bsel[:], d_bsel[:, :])
            nc.sync.dma_start(vbrow[:], d_vbrow[:, :])
            for i in range(2):
                cs = slice(128 * i, 128 * (i + 1))
                nc.sync.dma_start(xq[i][:], d_xq[cs, :])
            for i in range(2):
                cs = slice(128 * i, 128 * (i + 1))
                nc.sync.dma_start(pjt[i][:], d_pjT[cs, :])
            for i in range(2):
                csx = slice(128 * i, 128 * (i + 1))
                nc.sync.dma_start(xqf[i][:], d_xqf[csx, :])  # residual-only, not startup-critical

            # ones column for the softmax denominator (written once; the v
            # projection only writes cols 0..63, pad cols 65..79 never read)
            nc.gpsimd.memset(vt[:, :, :, :, 64:65], 1.0)

            # PSUM: 3x qk slots (2 banks each) + 2x av accumulators (1 bank each)
            with tc.tile_pool(name="psQK", bufs=3, space="PSUM") as psQK, \
                 tc.tile_pool(name="psACC", bufs=2, space="PSUM") as psACC:

                # ================= GroupNorm stats -> per-channel A, B ==========
                A = [small.tile([128, 1], F32, tag=f"A{i}", name=f"A{i}") for i in range(2)]
                Bs = [small.tile([128, 1], F32, tag=f"B{i}", name=f"B{i}") for i in range(2)]
                pcs = [small.tile([128, 2], F32, tag=f"pcs{i}", name=f"pcs{i}") for i in range(2)]
                # GN stats: bn_stats chunks in DMA-landing order (halves
                # interleaved) so the DVE pipeline starts on the first chunk
                statst = [work.tile([128, 8, 6], F32, tag=f"bnstats{i}",
                                    name=f"bnstats{i}") for i in range(2)]
                for j in range(8):
                    for i in range(2):
                        nc.vector.bn_stats(statst[i][:, j, :],
                                           x[i][:, 512 * j:512 * (j + 1)])
                for i in range(2):
                    mv = work.tile([128, 2], F32, tag="bnmv", name="bnmv")
                    nc.vector.bn_aggr(mv[:], statst[i][:])
                    nc.vector.tensor_copy(pcs[i][:, 0:1], mv[:, 0:1])
                    nc.vector.tensor_tensor(pcs[i][:, 1:2], mv[:, 0:1], mv[:, 0:1],
                                            mybir.AluOpType.mult)
                    nc.vector.tensor_tensor(pcs[i][:, 1:2], pcs[i][:, 1:2], mv[:, 1:2],
                                            mybir.AluOpType.add)

                # group stats [G, 2] = (mean_g, E[x^2]_g)
                grp_ps = psACC.tile([G, 2], F32, tag="av", name="s")
                nc.tensor.matmul(grp_ps[:], gsel[:, 0:G], pcs[0][:], start=True, stop=False)
                nc.tensor.matmul(grp_ps[:], gsel[:, G:2 * G], pcs[1][:], start=False, stop=True)

                grp_sb = small.tile([G, 2], F32, tag="grp_sb", name="grp_sb")
                nc.vector.tensor_copy(grp_sb[:], grp_ps[:])
                grp2 = small.tile([G, 2], F32, tag="grp2", name="grp2")  # (mean, rstd)
                var = small.tile([G, 1], F32, tag="var", name="var")
                epst = small.tile([G, 1], F32, tag="epst", name="epst")
                nc.vector.memset(epst[:], EPS)
                nc.vector.tensor_copy(grp2[:, 0:1], grp_sb[:, 0:1])
                nc.vector.tensor_tensor(var[:], grp_sb[:, 0:1], grp_sb[:, 0:1],
                                        mybir.AluOpType.mult)
                nc.vector.tensor_tensor(var[:], grp_sb[:, 1:2], var[:],
                                        mybir.AluOpType.subtract)
                nc.scalar.activation(var[:], var[:], mybir.ActivationFunctionType.Ln,
                                     bias=epst[:])
                nc.scalar.activation(grp2[:, 1:2], var[:],
                                     mybir.ActivationFunctionType.Exp, scale=-0.5)

                # broadcast to channels; A = rstd*gn_w, B = gn_b - mean*A
                for i in range(2):
                    mb_ps = psACC.tile([128, 2], F32, tag="av", name="s")
                    nc.tensor.matmul(mb_ps[:], bsel[:, 128 * i:128 * (i + 1)], grp2[:],
                                     start=True, stop=True)
                    nc.vector.tensor_tensor(A[i][:], mb_ps[:, 1:2], gnw[i][:],
                                            mybir.AluOpType.mult)
                    nc.vector.tensor_tensor(Bs[i][:], mb_ps[:, 0:1], A[i][:],
                                            mybir.AluOpType.mult)
                    nc.vector.tensor_tensor(Bs[i][:], gnb[i][:], Bs[i][:],
                                            mybir.AluOpType.subtract)
                    # folded weights
                    nc.vector.tensor_scalar_mul(wtf[i][:], wt[i][:], A[i][:])

                # folded q/k biases: qkvb[o] + sum_c wT[c,o]*B[c]
                for o in range(4):
                    b_ps = psACC.tile([128, 1], F32, tag="av", name="s")
                    nc.tensor.matmul(b_ps[:], wt[0][:, 128 * o:128 * (o + 1)], Bs[0][:],
                                     start=True, stop=False)
                    nc.tensor.matmul(b_ps[:], wt[1][:, 128 * o:128 * (o + 1)], Bs[1][:],
                                     start=False, stop=True)
                    nc.vector.tensor_tensor(qkb[o][:], smalls[:, 6 + o:7 + o], b_ps[:],
                                            mybir.AluOpType.add)
                # folded v bias as a row [1, C], then broadcast to 128 partitions
                vb_ps = psACC.tile([1, C], F32, tag="av", name="s")
                nc.tensor.matmul(vb_ps[:], Bs[0][:], wt[0][:, 2 * C:3 * C],
                                 start=True, stop=False)
                nc.tensor.matmul(vb_ps[:], Bs[1][:], wt[1][:, 2 * C:3 * C],
                                 start=False, stop=True)
                vb_row = small.tile([1, C], F32, tag="vb_row", name="vb_row")
                nc.vector.tensor_tensor(vb_row[:], vb_ps[:], vbrow[:],
                                        mybir.AluOpType.add)
                nc.gpsimd.partition_broadcast(vbias_bc[:], vb_row[:])

                # scaled q/k biases for the ScalarE epilogue path:
                # (x + b)*SCALE == Copy(SCALE*x + SCALE*b)
                qkbS = [small.tile([128, 1], F32, tag=f"qkbS{o}", name=f"qkbS{o}")
                        for o in range(4)]
                for o in range(4):
                    nc.vector.tensor_scalar_mul(qkbS[o][:], qkb[o][:], SCALE)

                # ================= q / k / vT production =====================
                def qk_pair(dst, wofs, src_t, bias_i, o, t, eng="D"):
                    # two t-tiles through one qk slot: 4 mms + 1 epilogue
                    ts2 = slice(512 * t, 512 * (t + 2))
                    ps = psQK.tile([128, 2, 512], F32, tag="qk", name="kps")
                    for u in range(2):
                        ts = slice(512 * (t + u), 512 * (t + u + 1))
                        nc.tensor.matmul(ps[:, u, :],
                                         wtf[0][:, wofs + 128 * o:wofs + 128 * (o + 1)],
                                         src_t[0][:, ts], start=True, stop=False)
                        nc.tensor.matmul(ps[:, u, :],
                                         wtf[1][:, wofs + 128 * o:wofs + 128 * (o + 1)],
                                         src_t[1][:, ts], start=False, stop=True)
                    dst_ap = dst[o][:, ts2].rearrange("p (u f) -> p u f", u=2)
                    if eng == "A":
                        nc.scalar.activation(dst_ap, ps[:],
                                             mybir.ActivationFunctionType.Identity,
                                             bias=qkbS[bias_i][:], scale=SCALE)
                    else:
                        nc.vector.tensor_scalar(dst_ap, ps[:], qkb[bias_i][:], SCALE,
                                                mybir.AluOpType.add,
                                                mybir.AluOpType.mult)

                def vt_pair(j):
                    # s-chunks 2j, 2j+1 -> vt[:, j, 0:2, :, 0:64]
                    ps = psQK.tile([128, 2, 512], F32, tag="qk", name="vps")[:, :, 0:256]
                    for u in range(2):
                        ss = slice(128 * (2 * j + u), 128 * (2 * j + u + 1))
                        nc.tensor.matmul(ps[:, u, :], x[0][:, ss], wtf[0][:, 2 * C:3 * C],
                                         start=True, stop=False)
                        nc.tensor.matmul(ps[:, u, :], x[1][:, ss], wtf[1][:, 2 * C:3 * C],
                                         start=False, stop=True)
                    nc.vector.tensor_tensor(
                        vt[:, j, :, :, 0:CH],
                        ps.rearrange("p u (h c) -> p u h c", h=NH),
                        bass.AP(tensor=vbias_bc[:].tensor, offset=vbias_bc[:].offset,
                                ap=[vbias_bc[:].ap[0], [0, 2], [CH, NH], [1, CH]]),
                        mybir.AluOpType.add)

                # minimal pre-work for unit (0,0): k heads 0-1, q heads 0-1
                # t 0..1023, first 4 vt chunk-pairs. The rest rides the units.
                # PRE epilogues go to ScalarE (it idles during startup).
                qk_pair(q_sb, 0, xq, 0, 0, 0, "A")
                for t in range(0, 8, 2):
                    qk_pair(k_sb, C, x, 2, 0, t, "A")
                for j in range(4):
                    vt_pair(j)

                # per-unit production hooks: {(p, tb): {scp: [fn, ...]}}
                hooks = {
                    (0, 0): {j - 3: [lambda j=j: vt_pair(j)] for j in range(4, NSCP)},
                    (0, 1): {3: [lambda: qk_pair(k_sb, C, x, 3, 1, 0, "A")],
                             8: [lambda: qk_pair(k_sb, C, x, 3, 1, 2, "D")],
                             12: [lambda: qk_pair(q_sb, 0, xq, 0, 0, 2, "A")]},
                    (0, 2): {3: [lambda: qk_pair(k_sb, C, x, 3, 1, 4, "D")],
                             8: [lambda: qk_pair(k_sb, C, x, 3, 1, 6, "A")],
                             12: [lambda: qk_pair(q_sb, 0, xq, 1, 1, 0, "D")]},
                    (0, 3): {8: [lambda: qk_pair(q_sb, 0, xq, 1, 1, 2, "A")]},
                }

                # ================= attention + proj ==========================
                with tc.tile_pool(name="pexp", bufs=8) as pexp, \
                     tc.tile_pool(name="nrm", bufs=4) as nrm, \
                     tc.tile_pool(name="outp", bufs=3) as outp:

                    def emit_exp(eng, pe, slot):
                        if eng == "A":
                            # same Schraudolph bits on ScalarE: walrus rejects
                            # fp8 ACT output, so write the e5m2 bits via int8.
                            # Relu clamps the negative-bits hazard to +0.0.
                            nc.scalar.activation(pe[:].bitcast(I8), slot[:],
                                                 mybir.ActivationFunctionType.Relu,
                                                 bias=b8t[:], scale=A8)
                        else:
                            nc.vector.tensor_scalar(pe[:].bitcast(I8), slot[:], A8, B8,
                                                    mybir.AluOpType.mult,
                                                    mybir.AluOpType.add)

                    def emit_avden(p, accs, hi, scp, pe):
                        h = 2 * p + hi
                        nc.tensor.matmul(accs[hi][0:65, :],
                                         vt[:, scp, :, h, 0:65],
                                         pe[:],
                                         start=(scp == 0), stop=(scp == NSCP - 1),
                                         perf_mode=mybir.MatmulPerfMode.DoubleRow)

                    def normalize(p, tb, accs):
                        tbs = slice(512 * tb, 512 * (tb + 1))
                        for hi in range(2):
                            hp = slice(64 * hi, 64 * hi + 64)
                            acc = accs[hi]
                            den = nrm.tile([1, 512], F32, tag="den", name="den")
                            nc.vector.tensor_copy(den[:], acc[64:65, :])
                            rec = nrm.tile([1, 512], F32, tag="rec", name="rec")
                            nc.vector.reciprocal_approx_fast(rec[:], den[:])
                            bc = nrm.tile([CH, 512], F32, tag="bc", name="bc")
                            nc.gpsimd.partition_broadcast(bc[:], rec[:])
                            # acc is PSUM -> GpSimd can't read it; DVE only
                            nc.vector.tensor_tensor(a_sb[p][hp, tbs], acc[0:CH, :],
                                                    bc[:], mybir.AluOpType.mult)

                    def proj_tb(tb):
                        tbs = slice(512 * tb, 512 * (tb + 1))
                        for o in range(2):
                            pr = psQK.tile([128, 512], F32, tag="qk", name="pr")
                            nc.tensor.matmul(pr[:], pjt[0][:, 128 * o:128 * (o + 1)],
                                             a_sb[0][:, tbs], start=True, stop=False)
                            nc.tensor.matmul(pr[:], pjt[1][:, 128 * o:128 * (o + 1)],
                                             a_sb[1][:, tbs], start=False, stop=True)
                            # PSUM+bias on ScalarE, then the SBUF-only residual
                            # add on GpSimd -- keeps VectorE free for exp
                            res = outp.tile([128, 512], F32, tag="res", name="res")
                            nc.scalar.activation(res[:], pr[:],
                                                 mybir.ActivationFunctionType.Identity,
                                                 bias=pjb[o][:])
                            nc.gpsimd.tensor_tensor(res[:], res[:], xqf[o][:, tbs],
                                                    mybir.AluOpType.add)
                            nc.sync.dma_start(d_out[128 * o:128 * (o + 1), tbs], res[:])

                    pattern_std = _eng_pattern()
                    # unit (0,0) carries 12 vt epilogues on VectorE -> shift
                    # its exp calls toward ScalarE
                    pattern_u0 = _eng_pattern({"A": 22, "D": 10})
                    units = [(p, tb) for p in range(2) for tb in range(THALF // 512)]
                    fin = None          # (p, tb, accs) awaiting normalize/proj
                    for (p, tb) in units:
                        pattern = pattern_u0 if (p, tb) == (0, 0) else pattern_std
                        tbs = slice(512 * tb, 512 * (tb + 1))
                        uhooks = hooks.get((p, tb), {})
                        accs = [psACC.tile([128, 512], F32, tag="av", name="av")
                                for _ in range(2)]
                        pend = deque()
                        ci = 0
                        for scp in range(NSCP):
                            # both heads' slots filled with interleaved MMs so
                            # consecutive MMs hit disjoint PE row groups and
                            # run concurrently
                            slots = [psQK.tile([128, 2, 512], F32, tag="qk",
                                               name="qk") for _ in range(2)]
                            for u in range(2):
                                sc = 2 * scp + u
                                for hi in range(2):
                                    hp = slice(64 * hi, 64 * hi + 64)
                                    nc.tensor.matmul(
                                        slots[hi][:, u, :],
                                        k_sb[p][hp, 128 * sc:128 * (sc + 1)],
                                        q_sb[p][hp, tbs],
                                        start=True, stop=True)
                            for hi in range(2):
                                pe = pexp.tile([128, 2, 512], F8E5, tag="pe", name="pe")
                                emit_exp(pattern[ci], pe, slots[hi])
                                ci += 1
                                pend.append((hi, scp, pe))
                                while len(pend) > 3:
                                    emit_avden(p, accs, *pend.popleft())
                            if scp == 0 and fin is not None:
                                # previous unit's epilogue, after this unit's QK
                                # stream has started feeding the engines
                                normalize(fin[0], fin[1], fin[2])
                            if scp == 1 and fin is not None:
                                if fin[0] == 1:
                                    proj_tb(fin[1])
                                fin = None
                            for fn_ in uhooks.get(scp, []):
                                fn_()
                        while pend:
                            emit_avden(p, accs, *pend.popleft())
                        fin = (p, tb, accs)
                    normalize(*fin)
                    proj_tb(fin[1])

    nc.compile()
    return nc


def _host_consts():
    g1 = np.zeros((128, G), dtype=np.float32)
    g2 = np.zeros((128, G), dtype=np.float32)
    for c in range(128):
        g1[c, c // CPG] = 1.0 / CPG
        g2[c, G // 2 + c // CPG] = 1.0 / CPG
    gsel = np.concatenate([g1, g2], axis=1)          # [128, 2G]
    bsel = np.zeros((G, C), dtype=np.float32)
    for c in range(C):
        bsel[c // CPG, c] = 1.0
    return gsel, bsel


def kernel(x, gn_w, gn_b, qkv_w, qkv_b, proj_w, proj_b):
    global LAST_RESULTS
    if "nc" not in _CACHE:
        _CACHE["nc"] = _build_program()
    nc = _CACHE["nc"]

    x = np.ascontiguousarray(np.asarray(x, dtype=np.float32))
    xr = x.reshape(B, C, T)
    gsel, bsel = _host_consts()
    smalls = np.zeros((128, 76), dtype=np.float32)
    smalls[:, 0:2] = np.asarray(gn_w, np.float32).reshape(2, 128).T
    smalls[:, 2:4] = np.asarray(gn_b, np.float32).reshape(2, 128).T
    smalls[:, 4:6] = np.asarray(proj_b, np.float32).reshape(2, 128).T
    smalls[:, 6:12] = np.asarray(qkv_b, np.float32).reshape(6, 128).T
    smalls[:, 12:76] = gsel
    shared = {
        "wT": np.ascontiguousarray(np.asarray(qkv_w, np.float32).T),
        "smalls": smalls,
        "vbrow": np.asarray(qkv_b, np.float32)[2 * C:].reshape(1, C).copy(),
        "pjT": np.ascontiguousarray(np.asarray(proj_w, np.float32).T.astype(np.float16)),
        "bsel": bsel,
    }
    in_maps = []
    for c in range(NCORES):
        b, hf = c // 2, c % 2
        m = dict(shared)
        m["xb"] = np.ascontiguousarray(xr[b].astype(np.float16))
        xqs = np.ascontiguousarray(xr[b][:, hf * THALF:(hf + 1) * THALF])
        m["xq"] = xqs.astype(np.float16)
        m["xqf"] = xqs
        in_maps.append(m)

    res = bass_utils.run_bass_kernel_spmd(nc, in_maps, core_ids=list(range(NCORES)))
    LAST_RESULTS = res

    out = np.empty((B, C, T), dtype=np.float32)
    for c in range(NCORES):
        b, hf = c // 2, c % 2
        out[b][:, hf * THALF:(hf + 1) * THALF] = res.results[c]["out"]
    return out.reshape(B, C, HH, WW)
